# revision 45
# baseline (speedup 1.0000x reference)
"""Trainium2 Bass kernel for ChamferLoss (B=8, C=3, N=4096), 8 NeuronCores.

Strategy: data-parallel over batch; core b computes batch b fully.
  D[n,m] = ||x_n||^2 + ||y_m||^2 - 2 x_n.y_m   (x = ori, y = adv points)
  d1 = mean_n relu(min_m D),  d2 = mean_m relu(min_n D)
Host combines: mean_b max(d1_b, d2_b).

One-pass design: D is computed once per slab of 128 rows (two [128, 2048]
fp32 PSUM units). The Scalar engine drains each unit to bf16 SBUF (ACT and
DVE are the only engines that can read PSUM, at 1 elem/cycle/partition, so
PSUM traffic is paid exactly once per element). The Vector engine then
  - computes the slab row-min with ONE custom fused DVE op
    (out = min(in0, in1), accum_out = min-reduce of out) — the stock
    TENSOR_TENSOR_REDUCE opcode's firmware table only implements
    mult/add, so a custom table op is registered instead, and
  - folds the slab into a ping-pong column-min accumulator with one
    tensor_tensor min.
The column direction finishes with 32 PE transposes of the accumulator
(bf16 stays bf16 into PSUM) + per-tile min reduces. relu is applied to
the [128, 32] partials at the end (relu commutes with min).

The -2*x.y matmul has contraction K=3; fp32 matmul is 4x slower on PE, so
each fp32 value v is split v = vh + vl (bf16 pair) and the product uses the
3-term expansion  x.y ~= xh.yh + xh.yl + xl.yh  (error ~2^-16 relative).
The squared norms are folded into the same matmul via constant-one rows, so
PSUM holds complete distance values. The [128, 4096] bf16 stationary/moving
images (13 used rows, replicated at partition offsets 32/64/96 so four PE
row-groups run concurrent matmuls via tile_position) are assembled on the
HOST in numpy and DMA'd in directly — this removes ~30us of on-device
prep/assembly ramp.
"""

import os
import sys

sys.path.insert(0, "/opt/trn_rl_repo")

import numpy as np
import ml_dtypes

import concourse.bass as bass  # noqa: F401  (registers engine types)
import concourse.tile as tile
from concourse import bacc, bass_utils, masks, mybir

B, C, N = 8, 3, 4096
NCORES = 8
NO = 32  # n_outer blocks of 128 rows
F32 = mybir.dt.float32
BF16 = mybir.dt.bfloat16
K = 13  # contraction rows: 9 coord product terms + 2 sq rows + 2 one rows
BIG = 3.0e38
BF = ml_dtypes.bfloat16
MINMIN_2X = bool(int(os.environ.get("MINMIN_2X", "1")))

_CACHE = {}


def _make_2x(u1x):
    """Hand-author the 2X_1PORT uop program for the fused min/min-reduce.

    At 2x the engine streams two packed bf16 per port per cycle, exposed
    as SRC_0/SRC_0_HI and SRC_1/SRC_1_HI crossbar lanes. Mirrors the
    generated 1x program's conventions (input lane k feeds delay lane
    k-1; DelayInp.PREV_ALU_OUT captures the preceding slice's result;
    the accumulate slice is MIN(CURR_ALU_OUT, PREV_ALU_OUT) with
    alu_out_a_enable, propagated to the last slice by BYPASS stages):

      slice0: lo  = min(src0_lo, src1_lo)
      slice1: hi  = min(src0_hi, src1_hi)   delay0 <- lo
      slice2: pr  = min(hi, lo)             delay1 <- hi
      slice3: acc = min(acc, pr)            (alu_out_a_enable)
      slice4-7: BYPASS acc propagation
      out: WR0_LO <- DELAY_0 (lo), WR0_HI <- DELAY_1 (hi)
    """
    import copy
    from concourse.dve_uop import AluInp, AluOp, DelayInp, InpSel, OutPath, OutSel

    u = copy.deepcopy(u1x)
    for uc in u:
        uc.inp[4] = InpSel.SRC_0_HI
        uc.inp[5] = InpSel.SRC_1_HI
        uc.inp_enable[4] = 1
        uc.inp_enable[5] = 1
    seed, steady = u
    PD, PA = DelayInp.PREV_DELAY, DelayInp.PREV_ALU_OUT

    def mk(op, s0, s1, delay, dly_en, a_en):
        d = copy.deepcopy(steady.datapath_config[0])
        d.op = op
        d.alu_src0 = s0
        d.alu_src1 = s1
        d.delay = delay
        d.delay_enable = dly_en
        d.alu_out_enable = 1
        d.alu_out_a_enable = a_en
        d.alu_out_b_enable = 0
        d.swap_enable = 0
        return d

    MIN, BYP = AluOp.MIN, AluOp.BYPASS
    en5 = [1, 1, 1, 1, 1, 0, 0]
    en3 = [1, 1, 1, 0, 0, 0, 0]
    # Scan layout mirroring the generated 1x program's conventions: the
    # scan state is a slice's own registered output (CURR_ALU_OUT), no
    # a/b registers, no ACCUM machinery. The generated seed BYPASSes the
    # C0 init through the whole chain, so slice3's output register
    # starts at C0. The out stream is the running min; the caller reads
    # its last column as the row minimum.
    steady.datapath_config = [
        mk(MIN, AluInp.PREV_DELAY_0, AluInp.PREV_DELAY_1,
           [PD, PD, PD, PD, PD, PA, PA], en5, 0),          # lo = min(s0, s1)
        mk(MIN, AluInp.PREV_ALU_OUT, AluInp.PREV_DELAY_3,
           [PD, PD, PD, PD, PD, PA, PA], en5, 0),          # u = min(lo, s0_hi)
        mk(MIN, AluInp.PREV_ALU_OUT, AluInp.PREV_DELAY_4,
           [PD, PD, PD, PD, PD, PA, PA], en5, 0),          # p = min(u, s1_hi)
        mk(MIN, AluInp.CURR_ALU_OUT, AluInp.PREV_ALU_OUT,
           [PD, PD, PD, PA, PA, PA, PA], en3, 0),          # s = min(s, p)
    ] + [
        mk(BYP, AluInp.PREV_ALU_OUT, AluInp.PREV_ALU_OUT,
           [PD, PD, PD, PA, PA, PA, PA], en3, 0)
        for _ in range(4)
    ]
    steady.out = {
        OutPath.WR0_LO: OutSel.ALU_OUT,
        OutPath.WR0_HI: OutSel.ALU_OUT,
        OutPath.WR1_LO: OutSel.ALU_OUT,
        OutPath.WR1_HI: OutSel.ALU_OUT,
    }
    steady.out_enable = {
        OutPath.WR0_LO: 1,
        OutPath.WR0_HI: 1,
        OutPath.WR1_LO: 0,
        OutPath.WR1_HI: 0,
    }
    return u


def _register_minmin_op():
    """Register the fused (min, min-reduce) custom DVE op at runtime.

    out = min(in0, in1); accum_out = min(s0, min over free dim of out).
    Uses the documented custom-DVE extension point (dve_ops.OPS +
    per-NEFF table gen); the sha is self-pinned since this op is defined
    here rather than in the repo's dve_ops registry.
    """
    if "minmin" in _CACHE:
        return _CACHE["minmin"]
    from concourse import dve_ops as dops
    from concourse.dve_spec import Spec, Src0, Src1, C0, minn, scan, lower, AluOp
    from concourse.dve_uop import DveOpSpec

    name = "CHAMFER_MINMIN_RED"

    def _ref(in0, in1, c0, c1, c2):
        m = np.minimum(in0, in1).astype(np.float32)
        s = np.minimum.accumulate(m.reshape(m.shape[0], -1), axis=1)
        return np.minimum(s, c0).reshape(m.shape)

    spec_ = Spec(
        body=scan(AluOp.MIN, minn(Src0, Src1), init=C0), reference=_ref
    )
    row = dops._CUSTOM_DVE_ROW_BASE + len(dops.OPS)

    class _MinMinOp:
        """Duck-typed DveOp: compile() attaches the hand-authored 2x
        program + perf_max so both codegen and the per-NEFF table carry
        the 2X_1PORT slot."""

        def __init__(self):
            self.name = name
            self.spec = spec_
            self.subdim = False
            self.perf_en = {}
            self.uops_sha = {}
            self._c = {}

        def compile(self, ver):
            if ver not in self._c:
                u1 = lower(spec_, ver=ver)
                s = DveOpSpec(name=self.name, opcode=row, uops=u1, rd1_en=True)
                if MINMIN_2X and ver == "v3":
                    s.uops_2x = _make_2x(u1)
                    s.perf_max = 1
                self._c[ver] = s
            return self._c[ver]

    op = _MinMinOp()
    dops.OPS.append(op)
    dops.CUSTOM_DVE_SPECS[name] = spec_
    dops._SUB_OPCODE_FOR_NAME[name] = row
    _CACHE["minmin"] = op
    return op


def _images(x: np.ndarray, y: np.ndarray):
    """Build the [128, 4096] bf16 lhsT/rhs images on the host.

    Row order (within each 32-partition quadrant replica t at offset 32t):
      lhs rows 0-2 = -2*xh_c, 3-5 = -2*xh_c, 6-8 = -2*xl_c,
          rows 9,10 = ones, rows 11,12 = x2h, x2l
      rhs rows 0-2 =    yh_c, 3-5 =    yl_c, 6-8 =    yh_c,
          rows 9,10 = y2h, y2l, rows 11,12 = ones
    so sum_k lhs[k,i]*rhs[k,j] = -2 x_i.y_j (3-term bf16 expansion)
    + ||y_j||^2 + ||x_i||^2.
    """

    def split(v):
        vh = v.astype(BF)
        vl = (v - vh.astype(np.float32)).astype(BF)
        return vh, vl

    def build(v, lhs):
        vh, vl = split(v)  # [3, N]
        v2 = (v * v).sum(axis=0)  # [N] fp32
        v2h, v2l = split(v2)
        img = np.zeros((128, N), dtype=BF)
        one = np.ones(N, dtype=BF)
        for t in range(4):
            o = 32 * t
            if lhs:
                m2h = (-2.0 * vh.astype(np.float32)).astype(BF)
                m2l = (-2.0 * vl.astype(np.float32)).astype(BF)
                img[o + 0 : o + 3] = m2h
                img[o + 3 : o + 6] = m2h
                img[o + 6 : o + 9] = m2l
                img[o + 9] = one
                img[o + 10] = one
                img[o + 11] = v2h
                img[o + 12] = v2l
            else:
                img[o + 0 : o + 3] = vh
                img[o + 3 : o + 6] = vl
                img[o + 6 : o + 9] = vh
                img[o + 9] = v2h
                img[o + 10] = v2l
                img[o + 11] = one
                img[o + 12] = one
        return img

    return build(x, lhs=True), build(y, lhs=False)


def _build():
    minmin = _register_minmin_op()
    nc = bacc.Bacc("TRN2", target_bir_lowering=False, debug=False)
    lx_d = nc.dram_tensor("lx", [128, N], BF16, kind="ExternalInput").ap()
    ry_d = nc.dram_tensor("ry", [128, N], BF16, kind="ExternalInput").ap()
    id_d = nc.dram_tensor("ident", [128, 128], BF16, kind="ExternalInput").ap()
    out_d = nc.dram_tensor("o", [128, 2], F32, kind="ExternalOutput").ap()

    with tile.TileContext(nc) as tc:
        with (
            tc.tile_pool(name="mats", bufs=1) as mats,
            tc.tile_pool(name="parts", bufs=1) as parts,
        ):
            LX = mats.tile([128, N], BF16, name="LX")
            RY = mats.tile([128, N], BF16, name="RY")
            # Split loads across queues, earliest-needed chunks first: the
            # first PSUM unit consumes RY[:, 0:2048], so that half rides
            # two queues in parallel; the identity (host-built, only
            # needed at the tail) trails the sync queue.
            nc.scalar.dma_start(LX[:, 0:512], lx_d[:, 0:512])
            nc.sync.dma_start(RY[:, 0:1024], ry_d[:, 0:1024])
            nc.gpsimd.dma_start(RY[:, 1024:2048], ry_d[:, 1024:2048])
            nc.scalar.dma_start(RY[:, 2048:4096], ry_d[:, 2048:4096])
            nc.scalar.dma_start(LX[:, 512:4096], lx_d[:, 512:4096])

            identity = parts.tile([128, 128], BF16)
            nc.sync.dma_start(identity[:], id_d[:])

            acc = [parts.tile([128, N], BF16, name=f"acc{i}") for i in range(2)]
            nc.vector.memset(acc[0][:], BIG)
            rowpart = parts.tile([128, NO], BF16)
            colpart = parts.tile([128, NO], F32)

            # Each slab r covers rows [128r, 128r+128) of D as two
            # [128, 2048] PSUM units, each filled by four concurrent PE
            # row-group matmuls (tile_position) using the replicated rows.
            def fill_unit(r, h):
                p = psum.tile([128, 2048], F32, name="pp")
                for j in range(4):
                    nc.tensor.matmul(
                        p[:, 512 * j : 512 * (j + 1)],
                        LX[32 * j : 32 * j + K, 128 * r : 128 * (r + 1)],
                        RY[32 * j : 32 * j + K,
                           2048 * h + 512 * j : 2048 * h + 512 * (j + 1)],
                        start=True,
                        stop=True,
                        tile_position=(32 * j, 0),
                    )
                return p

            with (
                tc.tile_pool(name="psum", bufs=2, space="PSUM") as psum,
                tc.tile_pool(name="drain", bufs=4) as drain,
                tc.tile_pool(name="scr", bufs=3) as scr,
            ):
                for r in range(NO):
                    c = drain.tile([128, N], BF16, name="c")
                    for h in range(2):
                        p = fill_unit(r, h)
                        nc.scalar.copy(c[:, 2048 * h : 2048 * (h + 1)], p[:])
                    scratch = scr.tile([128, 2048], BF16, name="scratch")
                    inst = nc.vector._custom_dve(
                        minmin,
                        out=scratch[:],
                        in0=c[:, 0:2048],
                        in1=c[:, 2048:4096],
                        s0=BIG,
                    )
                    if MINMIN_2X:
                        inst.ins.perf_max = 1
                    # The scan's last element is the row minimum; lift it
                    # out on otherwise-idle DMA queues before the scratch
                    # buffer rotates.
                    eng = nc.sync if r % 2 == 0 else nc.gpsimd
                    eng.dma_start(
                        rowpart[:, r : r + 1], scratch[:, 2047:2048]
                    )
                    nc.vector.tensor_tensor(
                        out=acc[(r + 1) % 2][:],
                        in0=acc[r % 2][:],
                        in1=c[:],
                        op=mybir.AluOpType.min,
                    )

            # Row-direction finals go first so they overlap the tail's
            # transposes on the PE.
            osb = parts.tile([128, 2], F32)
            nc.vector.tensor_scalar_max(rowpart[:], rowpart[:], 0.0)
            nc.vector.reduce_sum(osb[:, 0:1], rowpart[:], axis=mybir.AxisListType.X)

            # Tail: column minima. acc[p, m] = min over slabs; transpose
            # 128-column chunks (PE keeps bf16 into PSUM) and min-reduce
            # each to get colmin per column block.
            accf = acc[NO % 2]
            with tc.tile_pool(name="tpsum", bufs=8, space="PSUM") as tpsum:
                for k in range(NO):
                    tp = tpsum.tile([128, 128], BF16, name="tp")
                    nc.tensor.transpose(
                        tp[:], accf[:, 128 * k : 128 * (k + 1)], identity[:]
                    )
                    nc.vector.tensor_reduce(
                        colpart[:, k : k + 1],
                        tp[:],
                        axis=mybir.AxisListType.X,
                        op=mybir.AluOpType.min,
                    )

            nc.vector.tensor_scalar_max(colpart[:], colpart[:], 0.0)
            nc.vector.reduce_sum(osb[:, 1:2], colpart[:], axis=mybir.AxisListType.X)
            nc.sync.dma_start(out_d[:], osb[:])

    nc.compile()
    return nc


def kernel(ori_pcs: np.ndarray, adv_pcs: np.ndarray) -> np.ndarray:
    if "nc" not in _CACHE:
        _CACHE["nc"] = _build()
    nc = _CACHE["nc"]

    ori = np.ascontiguousarray(np.asarray(ori_pcs, dtype=np.float32))
    adv = np.ascontiguousarray(np.asarray(adv_pcs, dtype=np.float32))
    ident = np.eye(128, dtype=BF)
    in_maps = []
    for b in range(B):
        lx, ry = _images(ori[b], adv[b])
        in_maps.append({"lx": lx, "ry": ry, "ident": ident})
    res = bass_utils.run_bass_kernel_spmd(nc, in_maps, core_ids=list(range(NCORES)))

    vals = []
    for b in range(B):
        o = res.results[b]["o"].astype(np.float64)
        d1 = o[:, 0].sum() / N
        d2 = o[:, 1].sum() / N
        vals.append(max(d1, d2))
    return np.array(np.mean(vals), dtype=np.float32)


# revision 46
# speedup vs baseline: 1.1738x; 1.1738x over previous
"""Trainium2 Bass kernel for ChamferLoss (B=8, C=3, N=4096), 8 NeuronCores.

Strategy: data-parallel over batch; core b computes batch b fully.
  D[n,m] = ||x_n||^2 + ||y_m||^2 - 2 x_n.y_m   (x = ori, y = adv points)
  d1 = mean_n relu(min_m D),  d2 = mean_m relu(min_n D)
Host combines: mean_b max(d1_b, d2_b).

One-pass design: D is computed once per slab of 128 rows (two [128, 2048]
fp32 PSUM units). The Scalar engine drains each unit to bf16 SBUF (ACT and
DVE are the only engines that can read PSUM, at 1 elem/cycle/partition, so
PSUM traffic is paid exactly once per element). The Vector engine then
  - computes the slab row-min with ONE custom fused DVE op
    (out = min(in0, in1), accum_out = min-reduce of out) — the stock
    TENSOR_TENSOR_REDUCE opcode's firmware table only implements
    mult/add, so a custom table op is registered instead, and
  - folds the slab into a ping-pong column-min accumulator with one
    tensor_tensor min.
The column direction finishes with 32 PE transposes of the accumulator
(bf16 stays bf16 into PSUM) + per-tile min reduces. relu is applied to
the [128, 32] partials at the end (relu commutes with min).

The -2*x.y matmul has contraction K=3; fp32 matmul is 4x slower on PE, so
each fp32 value v is split v = vh + vl (bf16 pair) and the product uses the
3-term expansion  x.y ~= xh.yh + xh.yl + xl.yh  (error ~2^-16 relative).
The squared norms are folded into the same matmul via constant-one rows, so
PSUM holds complete distance values. The [128, 4096] bf16 stationary/moving
images (13 used rows, replicated at partition offsets 32/64/96 so four PE
row-groups run concurrent matmuls via tile_position) are assembled on the
HOST in numpy and DMA'd in directly — this removes ~30us of on-device
prep/assembly ramp.
"""

import os
import sys

sys.path.insert(0, "/opt/trn_rl_repo")

import numpy as np
import ml_dtypes

import concourse.bass as bass  # noqa: F401  (registers engine types)
import concourse.tile as tile
from concourse import bacc, bass_utils, masks, mybir

B, C, N = 8, 3, 4096
NCORES = 8
NO = 32  # n_outer blocks of 128 rows
F32 = mybir.dt.float32
BF16 = mybir.dt.bfloat16
K = 13  # contraction rows: 9 coord product terms + 2 sq rows + 2 one rows
BIG = 3.0e38
BF = ml_dtypes.bfloat16
MINMIN_2X = bool(int(os.environ.get("MINMIN_2X", "1")))

_CACHE = {}


def _make_2x(u1x):
    """Hand-author the 2X_1PORT uop program for the fused min/min-reduce.

    At 2x the engine streams two packed bf16 per port per cycle, exposed
    as SRC_0/SRC_0_HI and SRC_1/SRC_1_HI crossbar lanes. Mirrors the
    generated 1x program's conventions (input lane k feeds delay lane
    k-1; DelayInp.PREV_ALU_OUT captures the preceding slice's result;
    the accumulate slice is MIN(CURR_ALU_OUT, PREV_ALU_OUT) with
    alu_out_a_enable, propagated to the last slice by BYPASS stages):

      slice0: lo  = min(src0_lo, src1_lo)
      slice1: hi  = min(src0_hi, src1_hi)   delay0 <- lo
      slice2: pr  = min(hi, lo)             delay1 <- hi
      slice3: acc = min(acc, pr)            (alu_out_a_enable)
      slice4-7: BYPASS acc propagation
      out: WR0_LO <- DELAY_0 (lo), WR0_HI <- DELAY_1 (hi)
    """
    import copy
    from concourse.dve_uop import AluInp, AluOp, DelayInp, InpSel, OutPath, OutSel

    u = copy.deepcopy(u1x)
    for uc in u:
        uc.inp[4] = InpSel.SRC_0_HI
        uc.inp[5] = InpSel.SRC_1_HI
        uc.inp_enable[4] = 1
        uc.inp_enable[5] = 1
    seed, steady = u
    PD, PA = DelayInp.PREV_DELAY, DelayInp.PREV_ALU_OUT

    def mk(op, s0, s1, delay, dly_en, a_en):
        d = copy.deepcopy(steady.datapath_config[0])
        d.op = op
        d.alu_src0 = s0
        d.alu_src1 = s1
        d.delay = delay
        d.delay_enable = dly_en
        d.alu_out_enable = 1
        d.alu_out_a_enable = a_en
        d.alu_out_b_enable = 0
        d.swap_enable = 0
        return d

    MIN, BYP = AluOp.MIN, AluOp.BYPASS
    en5 = [1, 1, 1, 1, 1, 0, 0]
    en3 = [1, 1, 1, 0, 0, 0, 0]
    # Scan layout mirroring the generated 1x program's conventions: the
    # scan state is a slice's own registered output (CURR_ALU_OUT), no
    # a/b registers, no ACCUM machinery. The generated seed BYPASSes the
    # C0 init through the whole chain, so slice3's output register
    # starts at C0. The out stream is the running min; the caller reads
    # its last column as the row minimum.
    steady.datapath_config = [
        mk(MIN, AluInp.PREV_DELAY_0, AluInp.PREV_DELAY_1,
           [PD, PD, PD, PD, PD, PA, PA], en5, 0),          # lo = min(s0, s1)
        mk(MIN, AluInp.PREV_ALU_OUT, AluInp.PREV_DELAY_3,
           [PD, PD, PD, PD, PD, PA, PA], en5, 0),          # u = min(lo, s0_hi)
        mk(MIN, AluInp.PREV_ALU_OUT, AluInp.PREV_DELAY_4,
           [PD, PD, PD, PD, PD, PA, PA], en5, 0),          # p = min(u, s1_hi)
        mk(MIN, AluInp.CURR_ALU_OUT, AluInp.PREV_ALU_OUT,
           [PD, PD, PD, PA, PA, PA, PA], en3, 0),          # s = min(s, p)
    ] + [
        mk(BYP, AluInp.PREV_ALU_OUT, AluInp.PREV_ALU_OUT,
           [PD, PD, PD, PA, PA, PA, PA], en3, 0)
        for _ in range(4)
    ]
    steady.out = {
        OutPath.WR0_LO: OutSel.ALU_OUT,
        OutPath.WR0_HI: OutSel.ALU_OUT,
        OutPath.WR1_LO: OutSel.ALU_OUT,
        OutPath.WR1_HI: OutSel.ALU_OUT,
    }
    steady.out_enable = {
        OutPath.WR0_LO: 1,
        OutPath.WR0_HI: 1,
        OutPath.WR1_LO: 0,
        OutPath.WR1_HI: 0,
    }
    return u


def _register_minmin_op():
    """Register the fused (min, min-reduce) custom DVE op at runtime.

    out = min(in0, in1); accum_out = min(s0, min over free dim of out).
    Uses the documented custom-DVE extension point (dve_ops.OPS +
    per-NEFF table gen); the sha is self-pinned since this op is defined
    here rather than in the repo's dve_ops registry.
    """
    if "minmin" in _CACHE:
        return _CACHE["minmin"]
    from concourse import dve_ops as dops
    from concourse.dve_spec import Spec, Src0, Src1, C0, minn, scan, lower, AluOp
    from concourse.dve_uop import DveOpSpec

    name = "CHAMFER_MINMIN_RED"

    def _ref(in0, in1, c0, c1, c2):
        m = np.minimum(in0, in1).astype(np.float32)
        s = np.minimum.accumulate(m.reshape(m.shape[0], -1), axis=1)
        return np.minimum(s, c0).reshape(m.shape)

    spec_ = Spec(
        body=scan(AluOp.MIN, minn(Src0, Src1), init=C0), reference=_ref
    )
    row = dops._CUSTOM_DVE_ROW_BASE + len(dops.OPS)

    class _MinMinOp:
        """Duck-typed DveOp: compile() attaches the hand-authored 2x
        program + perf_max so both codegen and the per-NEFF table carry
        the 2X_1PORT slot."""

        def __init__(self):
            self.name = name
            self.spec = spec_
            self.subdim = False
            self.perf_en = {}
            self.uops_sha = {}
            self._c = {}

        def compile(self, ver):
            if ver not in self._c:
                u1 = lower(spec_, ver=ver)
                s = DveOpSpec(name=self.name, opcode=row, uops=u1, rd1_en=True)
                if MINMIN_2X and ver == "v3":
                    s.uops_2x = _make_2x(u1)
                    s.perf_max = 1
                self._c[ver] = s
            return self._c[ver]

    op = _MinMinOp()
    dops.OPS.append(op)
    dops.CUSTOM_DVE_SPECS[name] = spec_
    dops._SUB_OPCODE_FOR_NAME[name] = row
    _CACHE["minmin"] = op
    return op


def _images(x: np.ndarray, y: np.ndarray):
    """Build the [128, 4096] bf16 lhsT/rhs images on the host.

    Row order (within each 32-partition quadrant replica t at offset 32t):
      lhs rows 0-2 = -2*xh_c, 3-5 = -2*xh_c, 6-8 = -2*xl_c,
          rows 9,10 = ones, rows 11,12 = x2h, x2l
      rhs rows 0-2 =    yh_c, 3-5 =    yl_c, 6-8 =    yh_c,
          rows 9,10 = y2h, y2l, rows 11,12 = ones
    so sum_k lhs[k,i]*rhs[k,j] = -2 x_i.y_j (3-term bf16 expansion)
    + ||y_j||^2 + ||x_i||^2.
    """

    def split(v):
        vh = v.astype(BF)
        vl = (v - vh.astype(np.float32)).astype(BF)
        return vh, vl

    def build(v, lhs):
        vh, vl = split(v)  # [3, N]
        v2 = (v * v).sum(axis=0)  # [N] fp32
        v2h, v2l = split(v2)
        img = np.zeros((128, N), dtype=BF)
        one = np.ones(N, dtype=BF)
        for t in range(4):
            o = 32 * t
            if lhs:
                m2h = (-2.0 * vh.astype(np.float32)).astype(BF)
                m2l = (-2.0 * vl.astype(np.float32)).astype(BF)
                img[o + 0 : o + 3] = m2h
                img[o + 3 : o + 6] = m2h
                img[o + 6 : o + 9] = m2l
                img[o + 9] = one
                img[o + 10] = one
                img[o + 11] = v2h
                img[o + 12] = v2l
            else:
                img[o + 0 : o + 3] = vh
                img[o + 3 : o + 6] = vl
                img[o + 6 : o + 9] = vh
                img[o + 9] = v2h
                img[o + 10] = v2l
                img[o + 11] = one
                img[o + 12] = one
        return img

    return build(x, lhs=True), build(y, lhs=False)


def _build():
    minmin = _register_minmin_op()
    nc = bacc.Bacc("TRN2", target_bir_lowering=False, debug=False)
    lx_d = nc.dram_tensor("lx", [128, N], BF16, kind="ExternalInput").ap()
    ry_d = nc.dram_tensor("ry", [128, N], BF16, kind="ExternalInput").ap()
    id_d = nc.dram_tensor("ident", [128, 128], BF16, kind="ExternalInput").ap()
    out_d = nc.dram_tensor("o", [128, 2], F32, kind="ExternalOutput").ap()

    with tile.TileContext(nc) as tc:
        with (
            tc.tile_pool(name="mats", bufs=1) as mats,
            tc.tile_pool(name="parts", bufs=1) as parts,
        ):
            LX = mats.tile([128, N], BF16, name="LX")
            RY = mats.tile([128, N], BF16, name="RY")
            # Split loads across queues, earliest-needed chunks first: the
            # first PSUM unit consumes RY[:, 0:2048], so that half rides
            # two queues in parallel; the identity (host-built, only
            # needed at the tail) trails the sync queue.
            nc.sync.dma_start(LX[:, 0:512], lx_d[:, 0:512])
            nc.sync.dma_start(RY[:, 0:1024], ry_d[:, 0:1024])
            nc.gpsimd.dma_start(RY[:, 1024:2048], ry_d[:, 1024:2048])
            nc.scalar.dma_start(RY[:, 2048:4096], ry_d[:, 2048:4096])
            nc.scalar.dma_start(LX[:, 512:4096], lx_d[:, 512:4096])

            identity = parts.tile([128, 128], BF16)
            nc.sync.dma_start(identity[:], id_d[:])

            acc = [parts.tile([128, N], BF16, name=f"acc{i}") for i in range(2)]
            nc.vector.memset(acc[0][:], BIG)
            rowpart = parts.tile([128, NO], BF16)
            colpart = parts.tile([128, NO], F32)

            # Each slab r covers rows [128r, 128r+128) of D as two
            # [128, 2048] PSUM units, each filled by four concurrent PE
            # row-group matmuls (tile_position) using the replicated rows.
            def fill_unit(r, h):
                p = psum.tile([128, 2048], F32, name="pp")
                for j in range(4):
                    nc.tensor.matmul(
                        p[:, 512 * j : 512 * (j + 1)],
                        LX[32 * j : 32 * j + K, 128 * r : 128 * (r + 1)],
                        RY[32 * j : 32 * j + K,
                           2048 * h + 512 * j : 2048 * h + 512 * (j + 1)],
                        start=True,
                        stop=True,
                        tile_position=(32 * j, 0),
                    )
                return p

            with (
                tc.tile_pool(name="psum", bufs=2, space="PSUM") as psum,
                tc.tile_pool(name="drain", bufs=4) as drain,
                tc.tile_pool(name="scr", bufs=3) as scr,
            ):
                for r in range(NO):
                    c = drain.tile([128, N], BF16, name="c")
                    for h in range(2):
                        p = fill_unit(r, h)
                        nc.scalar.copy(c[:, 2048 * h : 2048 * (h + 1)], p[:])
                    scratch = scr.tile([128, 2048], BF16, name="scratch")
                    inst = nc.vector._custom_dve(
                        minmin,
                        out=scratch[:],
                        in0=c[:, 0:2048],
                        in1=c[:, 2048:4096],
                        s0=BIG,
                    )
                    if MINMIN_2X:
                        inst.ins.perf_max = 1
                    # The scan's last element is the row minimum; lift it
                    # out on otherwise-idle DMA queues before the scratch
                    # buffer rotates.
                    eng = nc.sync if r % 2 == 0 else nc.gpsimd
                    eng.dma_start(
                        rowpart[:, r : r + 1], scratch[:, 2047:2048]
                    )
                    nc.vector.tensor_tensor(
                        out=acc[(r + 1) % 2][:],
                        in0=acc[r % 2][:],
                        in1=c[:],
                        op=mybir.AluOpType.min,
                    )

            # Row-direction finals go first so they overlap the tail's
            # transposes on the PE.
            osb = parts.tile([128, 2], F32)
            nc.vector.tensor_scalar_max(rowpart[:], rowpart[:], 0.0)
            nc.vector.reduce_sum(osb[:, 0:1], rowpart[:], axis=mybir.AxisListType.X)

            # Tail: column minima. acc[p, m] = min over slabs; transpose
            # 128-column chunks (PE keeps bf16 into PSUM) and min-reduce
            # each to get colmin per column block.
            accf = acc[NO % 2]
            with tc.tile_pool(name="tpsum", bufs=8, space="PSUM") as tpsum:
                for k in range(NO):
                    tp = tpsum.tile([128, 128], BF16, name="tp")
                    nc.tensor.transpose(
                        tp[:], accf[:, 128 * k : 128 * (k + 1)], identity[:]
                    )
                    nc.vector.tensor_reduce(
                        colpart[:, k : k + 1],
                        tp[:],
                        axis=mybir.AxisListType.X,
                        op=mybir.AluOpType.min,
                    )

            nc.vector.tensor_scalar_max(colpart[:], colpart[:], 0.0)
            nc.vector.reduce_sum(osb[:, 1:2], colpart[:], axis=mybir.AxisListType.X)
            nc.sync.dma_start(out_d[:], osb[:])

    nc.compile()
    return nc


def kernel(ori_pcs: np.ndarray, adv_pcs: np.ndarray) -> np.ndarray:
    if "nc" not in _CACHE:
        _CACHE["nc"] = _build()
    nc = _CACHE["nc"]

    ori = np.ascontiguousarray(np.asarray(ori_pcs, dtype=np.float32))
    adv = np.ascontiguousarray(np.asarray(adv_pcs, dtype=np.float32))
    ident = np.eye(128, dtype=BF)
    in_maps = []
    for b in range(B):
        lx, ry = _images(ori[b], adv[b])
        in_maps.append({"lx": lx, "ry": ry, "ident": ident})
    res = bass_utils.run_bass_kernel_spmd(nc, in_maps, core_ids=list(range(NCORES)))

    vals = []
    for b in range(B):
        o = res.results[b]["o"].astype(np.float64)
        d1 = o[:, 0].sum() / N
        d2 = o[:, 1].sum() / N
        vals.append(max(d1, d2))
    return np.array(np.mean(vals), dtype=np.float32)
